# revision 17
# baseline (speedup 1.0000x reference)
"""Trainium2 Bass kernel for nn_PixelWiseAdpNet — v2.

Sharding: (batch=4) x (patch-row-half=2) -> 8 cores; each core owns one
batch's 4x8 block of patches (32 patches, 16384 points) and runs:
  phase A  : per-patch hyper-MLP params = w_feat @ F + b_feat, streamed
             as 89 slabs of [128, 2, 1024] bf16 with 7 in-flight DMA
             buffers (deep buffering is what keeps HBM near peak), c on
             partitions so param tiles are directly consumable as W^T
             stationary tiles; DVE drains add b_feat and pack pW1/pW2/pW3.
  fixup/L1 : the 1x1 coord conv is folded into L1.  Per-patch stationary
             statq [128, 256] = [W1^T | b1-row | garbage | (W1@w_cd)^T];
             moving xcoord [128 = em+b_cd | ones | zeros | cd, 512 pts].
             One K=128 matmul pair per patch replaces inX matmuls, the
             em+cd DVE adds, and lets L1's lrelu run bias-free as ONE
             ACT instruction (FD=1024).  b1 rows come from 2 batched PE
             transposes -> b1T, then a tiny SBUF->SBUF DMA per patch.
  L2/L3    : as v1 (h0 early under the stream, h1 late, L3 qa/qb pairs
             col-packed into one PSUM tile); L3 drains on DVE
             (tensor_scalar_add + b3), output stored bf16 (host upcasts).
All matmul inputs bf16 (fp32 PSUM accumulation).  ~170 us HW exec
(device-state dependent), absmax rel err ~6e-3 (bf16).
"""

import numpy as np
import ml_dtypes

import concourse.mybir as mybir
import concourse.tile as tile
from concourse import bacc
from concourse.bass_utils import run_bass_kernel_spmd
from concourse.masks import make_identity

BF16 = ml_dtypes.bfloat16

B, IN_CH, OUT_CH, FEAT_CH = 4, 32, 64, 256
AH = AW = 8
OUT_H = OUT_W = 64
S = 8
NEG = 0.01
C_TOTAL = 90688
C_PAD = 90752            # padded to a whole number of 128-c tiles
N_CORES = 8
N_TILES = C_PAD // 128   # 709
NQ = 32                  # patches per core

# tile indices in the REORDERED + padded c stream
#   [b1:0-1][b2:2-3][b3+pad:4][W1':5-68][W2:69-580][W3:581-708]
T_W1 = 5
T_W2 = 69
T_W3 = 581

SLAB_C = 1024            # c columns per w_feat DMA slab
WBUFS = 7                # w_feat stream buffers (in-flight DMA depth)
N_SLABS = (C_PAD + SLAB_C - 1) // SLAB_C
T_PER = SLAB_C // 128    # c tiles per slab
SL_FIX = 68 // T_PER + 1   # slab after which W1' region is complete
SL_L2 = 324 // T_PER + 1   # slab after which W2 h0 halves are complete
SL_H1 = 580 // T_PER + 1   # slab after which all of W2 is complete
H1E = 16                 # x2h1 bufs; L2h1 beyond this interleaves L3
F32 = mybir.dt.float32
BF = mybir.dt.bfloat16

_CACHE = {}


def _build(variant="all"):
    nreps = 1
    if variant.startswith("rep"):
        rep, _, sub = variant[3:].partition("_")
        nreps = int(rep)
        variant = sub or "all"
    nc = bacc.Bacc("TRN2", target_bir_lowering=False, debug=False,
                   num_devices=N_CORES)

    wfeatT_d = nc.dram_tensor("wfeatT", [2, 128, C_PAD], BF, kind="ExternalInput")
    bfeat_d = nc.dram_tensor("bfeat", [128, N_TILES], F32, kind="ExternalInput")
    mlpfT_d = nc.dram_tensor("mlpfT", [128, 2, NQ], BF, kind="ExternalInput")
    xcoord_d = nc.dram_tensor("xcoord", [8, 128, S, 8, 32], BF, kind="ExternalInput")
    wcde_d = nc.dram_tensor("wcde", [IN_CH, OUT_CH], BF, kind="ExternalInput")
    out_d = nc.dram_tensor("out", [4, 2, OUT_CH, S, 8, 32], BF, kind="ExternalOutput")

    with tile.TileContext(nc) as tc:
        with (
            tc.tile_pool(name="const", bufs=1) as const_pool,
            tc.tile_pool(name="wstream", bufs=WBUFS) as wpool,
            tc.tile_pool(name="params", bufs=1) as ppool,
            tc.tile_pool(name="xc", bufs=2) as xcpool,
            tc.tile_pool(name="statq", bufs=4) as spool,
            tc.tile_pool(name="acts", bufs=3) as apool,
            tc.tile_pool(name="late", bufs=1) as lpool,
            tc.tile_pool(name="psA", bufs=2, space="PSUM") as psA,
            tc.tile_pool(name="psL1", bufs=2, space="PSUM") as psL1,
            tc.tile_pool(name="psY", bufs=2, space="PSUM") as psY,
        ):
            for _rep in range(nreps):
                # ---- constants ----
                ident = const_pool.tile([128, 128], BF, name="ident")
                make_identity(nc, ident[:])
                mlpfT = const_pool.tile([128, 2, NQ], BF, name="mlpfT")
                nc.sync.dma_start(mlpfT[:], mlpfT_d[:])
                wcde = const_pool.tile([IN_CH, OUT_CH], BF, name="wcde")
                nc.sync.dma_start(wcde[:], wcde_d[:])
                bfeat = const_pool.tile([128, N_TILES], F32, name="bfeat")
                nc.sync.dma_start(bfeat[:], bfeat_d[:])

                pW1 = ppool.tile([128, T_W2 - T_W1, NQ], BF, name="pW1")
                pW2 = ppool.tile([128, NQ, T_W3 - T_W2], BF, name="pW2")
                pW3 = ppool.tile([128, NQ, N_TILES - T_W3], BF, name="pW3")
                bias_sb = ppool.tile([128, 5, 4, 8], F32, name="bias_sb")
                b1T = ppool.tile([NQ, 2, 128], BF, name="b1T")

                if variant == "noA":
                    for pt in (pW1, pW2, pW3):
                        nc.vector.memset(pt[:], 0.0)
                    nc.vector.memset(bias_sb[:], 0.0)
                    nc.vector.memset(b1T[:], 0.0)

                n_slabs = 0 if variant == "noA" else N_SLABS

                def emit_slab(sl, dma_only=False):
                    if not n_slabs:
                        return
                    c0 = sl * SLAB_C
                    cw = min(SLAB_C, C_PAD - c0)
                    if cw <= 0:
                        return
                    t0 = c0 // 128
                    ntile_sl = cw // 128
                    wbuf = wpool.tile([128, 2, SLAB_C], BF, name="wbuf")
                    for k in range(2):
                        nc.sync.dma_start(wbuf[:, k, :cw],
                                          wfeatT_d[k, :, c0:c0 + cw])
                    if dma_only:
                        return
                    ps = psA.tile([128, SLAB_C // 128, NQ], F32, name="ps")
                    for u in range(ntile_sl):
                        for k in range(2):
                            nc.tensor.matmul(
                                ps[:, u, :],
                                wbuf[:, k, u * 128:(u + 1) * 128],
                                mlpfT[:, k, :],
                                start=(k == 0), stop=(k == 1))
                    # drains: bias tiles -> bias_sb only; W regions -> pW*
                    u = 0
                    while u < ntile_sl:
                        t = t0 + u
                        if t < T_W1:
                            nc.vector.tensor_scalar_add(
                                bias_sb[:, t, :, :].opt(),
                                ps[:, u, :],
                                bfeat[:, t:t + 1])
                            u += 1
                            continue
                        if t < T_W2:
                            seg = min(ntile_sl - u, T_W2 - t)
                            nc.vector.tensor_tensor(
                                out=pW1[:, t - T_W1:t - T_W1 + seg, :],
                                in0=ps[:, u:u + seg, :],
                                in1=bfeat[:, t:t + seg].unsqueeze(2)
                                .broadcast_to((128, seg, NQ)),
                                op=mybir.AluOpType.add)
                            u += seg
                            continue
                        pt, lo = (pW2, T_W2) if t < T_W3 else (pW3, T_W3)
                        hi = T_W3 if t < T_W3 else N_TILES
                        seg = min(ntile_sl - u, hi - t)
                        nc.vector.tensor_tensor(
                            out=pt[:, :, t - lo:t - lo + seg],
                            in0=ps[:, u:u + seg, :].transpose([0, 2, 1]),
                            in1=bfeat[:, t:t + seg].unsqueeze(1)
                            .broadcast_to((128, NQ, seg)),
                            op=mybir.AluOpType.add)
                        u += seg

                if variant == "dmaonly":
                    for sl in range(N_SLABS):
                        emit_slab(sl, dma_only=True)
                    for g in range(8):
                        xcd = xcpool.tile([128, S, 8, 32], BF, name="xc")
                        nc.scalar.dma_start(xcd[:], xcoord_d[g])
                    for r in range(4):
                        out_row = lpool.tile([128, S, 8, 32], BF,
                                             name="out_rowD", bufs=2)
                        nc.vector.memset(out_row[:], 0.0)
                        for half in range(2):
                            nc.sync.dma_start(out_d[r, half], out_row[64 * half:64 * half + 64, :, :, 0:32])
                    continue

                mlp_on = variant != "nomlp"
                x1s, x2h0s, x2h1s = {}, {}, {}

                # ---- stream head: biases + W1' ----
                for sl in range(0, 1):
                    emit_slab(sl)

                if mlp_on:
                    # b1 rows for all patches: [128(h-part), 4, 8] -> [32, 128]
                    b12 = ppool.tile([128, 2, 4, 8], BF, name="b12")
                    nc.vector.tensor_copy(b12[:], bias_sb[:, 0:2, :, :])
                    psB = psY.tile([NQ, 2, 128], BF, name="psB", tag="y")
                    for h in range(2):
                        nc.tensor.transpose(psB[:, h, :],
                                            b12[:, h, :, :], ident[:])
                    nc.vector.tensor_copy(b1T[:], psB[:])

                for sl in range(1, SL_FIX):
                    emit_slab(sl)

                xcs = {}

                def load_xc(g):
                    xc = xcpool.tile([128, S, 8, 32], BF, name="xc")
                    nc.scalar.dma_start(xc[:], xcoord_d[g])
                    xcs[g] = xc

                def emit_fixup_l1(q):
                    wp = q % 8
                    # statq rows: [0:32 W1^T][32 b1][33:64 garbage][64:128
                    # (W1@w_cd)^T]; moving rows: [em+b_cd][ones][zeros][cd]
                    statq = spool.tile([128, 256], BF, name="statq")
                    if q < 4:  # zero garbage rows once per pool buffer
                        nc.vector.memset(statq[32:64, :], 0.0)
                    nc.scalar.dma_start(statq[32:33, :], b1T[q:q + 1, :, :])
                    psT = psY.tile([32, 256], BF, name="psT", tag="y")
                    for h in range(2):
                        nc.tensor.transpose(
                            psT[:, 128 * h:128 * h + 128],
                            pW1[:, h:h + 63:2, q], ident[:])
                    nc.vector.tensor_copy(statq[0:32, :], psT[:])
                    psP = psY.tile([128, 256], F32, name="psP", tag="y")
                    nc.tensor.matmul(psP[64:128, :], wcde[:], statq[0:32, :],
                                     start=True, stop=True,
                                     tile_position=(0, 64))
                    nc.vector.tensor_copy(statq[64:128, :], psP[64:128, :])
                    y1 = psL1.tile([128, 2, 512], F32, name="y1")
                    xq = xcs[q // 4][:, :, :, 8 * (wp % 4):8 * (wp % 4) + 8]
                    for h in range(2):
                        nc.tensor.matmul(y1[:, h, :],
                                         statq[:, 128 * h:128 * h + 128],
                                         xq, start=True, stop=True)
                    x1 = apool.tile([128, 2, 512], BF, name="x1", bufs=NQ)
                    nc.scalar.activation(
                        x1[:], y1[:], mybir.ActivationFunctionType.Lrelu,
                        scale=1.0, alpha=NEG)
                    x1s[q] = x1

                def emit_l2(q, h, store):
                    x2 = lpool.tile([128, 512], BF, name=f"x2h{h}",
                                    bufs=NQ if h == 0 else H1E)
                    y2 = psY.tile([128, 512], F32, name="y2", tag="y")
                    for k in range(2):
                        nc.tensor.matmul(
                            y2[:], pW2[:, q, 256 * h + 128 * k:
                                        256 * h + 128 * k + 128],
                            x1s[q][:, k, :], start=(k == 0), stop=(k == 1))
                    nc.scalar.activation(
                        x2[:], y2[:], mybir.ActivationFunctionType.Lrelu,
                        bias=bias_sb[:, 2 + h, q // 8, q % 8:q % 8 + 1],
                        scale=1.0, alpha=NEG)
                    store[q] = x2

                def spread(n_items, lo, hi):
                    nsl = hi - lo
                    return {lo + i: list(range(n_items * i // nsl,
                                               n_items * (i + 1) // nsl))
                            for i in range(nsl)}

                fix_sched = spread(NQ, SL_FIX, SL_L2)
                l20_sched = spread(NQ, SL_L2, SL_H1)
                l21_sched = spread(H1E, SL_H1, N_SLABS)

                if mlp_on:
                    load_xc(0)
                    load_xc(1)
                for sl in range(SL_FIX, N_SLABS):
                    emit_slab(sl)
                    if not mlp_on:
                        continue
                    for q in fix_sched.get(sl, ()):
                        if q % 4 == 0 and q // 4 + 2 <= 7:
                            load_xc(q // 4 + 2)
                        emit_fixup_l1(q)
                    for q in l20_sched.get(sl, ()):
                        emit_l2(q, 0, x2h0s)
                    for q in l21_sched.get(sl, ()):
                        emit_l2(q, 1, x2h1s)

                out_state = {}
                bias3all = lpool.tile([128, 4, 4], F32, name="bias3all")
                nc.sync.dma_start(bias3all[0:64, :, :],
                                  bias_sb[0:64, 4, :, 0:4])
                nc.sync.dma_start(bias3all[64:128, :, :],
                                  bias_sb[0:64, 4, :, 4:8])

                def emit_l3(pr):
                    r, po = pr // 4, pr % 4
                    qa, qb = 8 * r + po, 8 * r + po + 4
                    if po == 0:
                        out_state[r] = lpool.tile([128, S, 8, 32], BF,
                                                  name="out_row", bufs=2)
                    y3 = psY.tile([128, S, 8, 8], F32, name="y3", tag="y")
                    for k in range(2):
                        nc.tensor.matmul(
                            y3[0:64, :, :, :],
                            pW3[:, qa, 64 * k:64 * k + 64],
                            x2h0s[qa][:] if k == 0 else x2h1s[qa][:],
                            start=(k == 0), stop=(k == 1))
                    for k in range(2):
                        nc.tensor.matmul(
                            y3[64:128, :, :, :],
                            pW3[:, qb, 64 * k:64 * k + 64],
                            x2h0s[qb][:] if k == 0 else x2h1s[qb][:],
                            start=(k == 0), stop=(k == 1),
                            tile_position=(0, 64))
                    nc.vector.tensor_scalar_add(
                        out_state[r][:, :, :, 8 * po:8 * po + 8], y3[:],
                        bias3all[:, r, po:po + 1])
                    if po == 3:
                        for half in range(2):
                            nc.scalar.dma_start(
                                out_d[r, half],
                                out_state[r][64 * half:64 * half + 64,
                                             :, :, :])

                if variant == "nomlp":
                    for r in range(4):
                        out_row = lpool.tile([128, S, 8, 32], BF,
                                             name="out_rowM", bufs=2)
                        nc.vector.memset(out_row[:], 0.0)
                        for half in range(2):
                            nc.sync.dma_start(out_d[r, half], out_row[64 * half:64 * half + 64])
                if mlp_on:
                    # before l2h1(H1E+i) (slot i), the L3 pair consuming
                    # x2h1[i] must already be emitted (FIFO tag queues)
                    next_pr = [0]

                    def l3_upto(need):
                        while next_pr[0] <= need:
                            emit_l3(next_pr[0])
                            next_pr[0] += 1

                    for i in range(NQ - H1E):
                        l3_upto(4 * (i // 8) + (i % 8) % 4)
                        emit_l2(H1E + i, 1, x2h1s)
                    for pr in range(next_pr[0], NQ // 2):
                        emit_l3(pr)

    nc.compile()
    return nc


def _host_prep(MLP_feature, coord_em, coord_data, w_cd, b_cd, w_feat, b_feat):
    # build the reordered + padded stream:
    # [b1 256][b2 256][b3 64][pad 64][W1' 8192 (c'=i*256+o)][W2][W3]
    j = np.arange(8192)
    w1_perm = (j % 256) * 32 + (j // 256)           # orig c of W1' position j
    w_feat_r = np.zeros((C_PAD, FEAT_CH), np.float32)
    b_feat_r = np.zeros(C_PAD, np.float32)

    def put(dst0, src_idx):
        w_feat_r[dst0:dst0 + len(src_idx)] = w_feat[src_idx]
        b_feat_r[dst0:dst0 + len(src_idx)] = b_feat[src_idx]

    put(0, np.arange(8192, 8448))          # b1
    put(256, np.arange(73984, 74240))      # b2
    put(512, np.arange(90624, 90688))      # b3
    put(640, w1_perm)                      # W1'
    jw2 = np.arange(65536)
    t2, p2 = jw2 // 128, jw2 % 128
    h2, k2, o2 = t2 // 256, (t2 // 128) % 2, t2 % 128
    put(8832, (128 * h2 + o2) * 256 + 128 * k2 + p2 + 8448)      # W2 (h,k,o)
    jw3 = np.arange(16384)
    t3, p3 = jw3 // 128, jw3 % 128
    k3, o3 = t3 // 64, t3 % 64
    put(74368, o3 * 256 + 128 * k3 + p3 + 74240)                 # W3 (k,o)

    wfeatT = np.ascontiguousarray(
        w_feat_r.T.astype(BF16).reshape(2, 128, C_PAD))
    bfeat_t = np.ascontiguousarray(b_feat_r.reshape(N_TILES, 128).T)

    wcde = w_cd.astype(BF16)

    in_maps = []
    for core in range(N_CORES):
        b, hh = core // 2, core % 2
        mlpfT = np.ascontiguousarray(
            MLP_feature[b, :, 4 * hh:4 * hh + 4, :].reshape(2, 128, NQ)
            .transpose(1, 0, 2)).astype(BF16)
        em = coord_em[b].reshape(IN_CH, S, OUT_H, OUT_W)[
            :, :, 32 * hh:32 * hh + 32, :]
        cd = coord_data[b].reshape(S, OUT_H, OUT_W, OUT_CH)[
            :, 32 * hh:32 * hh + 32]
        xcoord = np.zeros((128, S, 32, OUT_W), BF16)
        xcoord[:32] = (em + b_cd[:, None, None, None]).astype(BF16)
        xcoord[32] = 1.0
        xcoord[64:] = cd.transpose(3, 0, 1, 2).astype(BF16)
        xcg = np.empty((8, 128, S, 8, 32), BF16)
        for g in range(8):
            r, wh = g // 2, g % 2
            xcg[g] = xcoord[:, :, 8 * r:8 * r + 8, 32 * wh:32 * wh + 32]
        in_maps.append({
            "wfeatT": wfeatT, "bfeat": bfeat_t, "mlpfT": mlpfT,
            "xcoord": xcg, "wcde": wcde,
        })
    return in_maps


def kernel(**inputs):
    inputs = {k: np.asarray(v) for k, v in inputs.items()}
    if "nc" not in _CACHE:
        _CACHE["nc"] = _build()
    nc = _CACHE["nc"]
    in_maps = _host_prep(**inputs)
    res = run_bass_kernel_spmd(nc, in_maps, core_ids=list(range(N_CORES)))
    out = np.empty((B, OUT_CH, S, OUT_H, OUT_W), np.float32)
    for core in range(N_CORES):
        b, hh = core // 2, core % 2
        o = res.results[core]["out"].astype(np.float32)  # [4, 2, 64, S, 8, 32]
        for r in range(4):
            for half in range(2):
                out[b, :, :, 32 * hh + 8 * r:32 * hh + 8 * r + 8,
                    32 * half:32 * half + 32] = o[r, half]
    return out


# revision 18
# speedup vs baseline: 1.4353x; 1.4353x over previous
"""Trainium2 Bass kernel for nn_PixelWiseAdpNet — v2.

Sharding: (batch=4) x (patch-row-half=2) -> 8 cores; each core owns one
batch's 4x8 block of patches (32 patches, 16384 points) and runs:
  phase A  : per-patch hyper-MLP params = w_feat @ F + b_feat, streamed
             as 89 slabs of [128, 2, 1024] bf16 with 7 in-flight DMA
             buffers (deep buffering is what keeps HBM near peak), c on
             partitions so param tiles are directly consumable as W^T
             stationary tiles; DVE drains add b_feat and pack pW1/pW2/pW3.
  fixup/L1 : the 1x1 coord conv is folded into L1.  Per-patch stationary
             statq [128, 256] = [W1^T | b1-row | garbage | (W1@w_cd)^T];
             moving xcoord [128 = em+b_cd | ones | zeros | cd, 512 pts].
             One K=128 matmul pair per patch replaces inX matmuls, the
             em+cd DVE adds, and lets L1's lrelu run bias-free as ONE
             ACT instruction (FD=1024).  b1 rows come from 2 batched PE
             transposes -> b1T, then a tiny SBUF->SBUF DMA per patch.
  L2/L3    : as v1 (h0 early under the stream, h1 late, L3 qa/qb pairs
             col-packed into one PSUM tile); L3 drains on DVE
             (tensor_scalar_add + b3), output stored bf16 (host upcasts).
All matmul inputs bf16 (fp32 PSUM accumulation).  ~170 us HW exec
(device-state dependent), absmax rel err ~6e-3 (bf16).
"""

import numpy as np
import ml_dtypes

import concourse.mybir as mybir
import concourse.tile as tile
from concourse import bacc
from concourse.bass_utils import run_bass_kernel_spmd
from concourse.masks import make_identity

BF16 = ml_dtypes.bfloat16

B, IN_CH, OUT_CH, FEAT_CH = 4, 32, 64, 256
AH = AW = 8
OUT_H = OUT_W = 64
S = 8
NEG = 0.01
C_TOTAL = 90688
C_PAD = 90752            # padded to a whole number of 128-c tiles
N_CORES = 8
N_TILES = C_PAD // 128   # 709
NQ = 32                  # patches per core

# tile indices in the REORDERED + padded c stream
#   [b1:0-1][b2:2-3][b3+pad:4][W1':5-68][W2:69-580][W3:581-708]
T_W1 = 5
T_W2 = 69
T_W3 = 581

SLAB_C = 1024            # c columns per w_feat DMA slab
WBUFS = 7                # w_feat stream buffers (in-flight DMA depth)
N_SLABS = (C_PAD + SLAB_C - 1) // SLAB_C
T_PER = SLAB_C // 128    # c tiles per slab
SL_FIX = 68 // T_PER + 1   # slab after which W1' region is complete
SL_L2 = 324 // T_PER + 1   # slab after which W2 h0 halves are complete
SL_H1 = 580 // T_PER + 1   # slab after which all of W2 is complete
H1E = 16                 # x2h1 bufs; L2h1 beyond this interleaves L3
F32 = mybir.dt.float32
BF = mybir.dt.bfloat16

_CACHE = {}


def _build(variant="all"):
    nreps = 1
    if variant.startswith("rep"):
        rep, _, sub = variant[3:].partition("_")
        nreps = int(rep)
        variant = sub or "all"
    nc = bacc.Bacc("TRN2", target_bir_lowering=False, debug=False,
                   num_devices=N_CORES)

    wfeatT_d = nc.dram_tensor("wfeatT", [2, 128, C_PAD], BF, kind="ExternalInput")
    bfeat_d = nc.dram_tensor("bfeat", [128, N_TILES], F32, kind="ExternalInput")
    mlpfT_d = nc.dram_tensor("mlpfT", [128, 2, NQ], BF, kind="ExternalInput")
    xcoord_d = nc.dram_tensor("xcoord", [8, 128, S, 8, 32], BF, kind="ExternalInput")
    wcde_d = nc.dram_tensor("wcde", [IN_CH, OUT_CH], BF, kind="ExternalInput")
    out_d = nc.dram_tensor("out", [4, 2, OUT_CH, S, 8, 32], BF, kind="ExternalOutput")

    with tile.TileContext(nc) as tc:
        with (
            tc.tile_pool(name="const", bufs=1) as const_pool,
            tc.tile_pool(name="wstream", bufs=WBUFS) as wpool,
            tc.tile_pool(name="params", bufs=1) as ppool,
            tc.tile_pool(name="xc", bufs=2) as xcpool,
            tc.tile_pool(name="statq", bufs=6) as spool,
            tc.tile_pool(name="acts", bufs=3) as apool,
            tc.tile_pool(name="late", bufs=1) as lpool,
            tc.tile_pool(name="psA", bufs=2, space="PSUM") as psA,
            tc.tile_pool(name="psL1", bufs=2, space="PSUM") as psL1,
            tc.tile_pool(name="psY", bufs=2, space="PSUM") as psY,
        ):
            for _rep in range(nreps):
                # ---- constants ----
                ident = const_pool.tile([128, 128], BF, name="ident")
                make_identity(nc, ident[:])
                mlpfT = const_pool.tile([128, 2, NQ], BF, name="mlpfT")
                nc.sync.dma_start(mlpfT[:], mlpfT_d[:])
                wcde = const_pool.tile([IN_CH, OUT_CH], BF, name="wcde")
                nc.sync.dma_start(wcde[:], wcde_d[:])
                bfeat = const_pool.tile([128, N_TILES], F32, name="bfeat")
                nc.sync.dma_start(bfeat[:], bfeat_d[:])

                pW1 = ppool.tile([128, T_W2 - T_W1, NQ], BF, name="pW1")
                pW2 = ppool.tile([128, NQ, T_W3 - T_W2], BF, name="pW2")
                pW3 = ppool.tile([128, NQ, N_TILES - T_W3], BF, name="pW3")
                bias_sb = ppool.tile([128, 5, 4, 8], F32, name="bias_sb")
                b1T = ppool.tile([NQ, 2, 128], BF, name="b1T")

                if variant == "noA":
                    for pt in (pW1, pW2, pW3):
                        nc.vector.memset(pt[:], 0.0)
                    nc.vector.memset(bias_sb[:], 0.0)
                    nc.vector.memset(b1T[:], 0.0)

                n_slabs = 0 if variant == "noA" else N_SLABS
                ps_pair = {}

                def emit_slab(sl, dma_only=False):
                    if not n_slabs:
                        return
                    c0 = sl * SLAB_C
                    cw = min(SLAB_C, C_PAD - c0)
                    if cw <= 0:
                        return
                    t0 = c0 // 128
                    ntile_sl = cw // 128
                    wbuf = wpool.tile([128, 2, SLAB_C], BF, name="wbuf")
                    for k in range(2):
                        nc.sync.dma_start(wbuf[:, k, :cw],
                                          wfeatT_d[k, :, c0:c0 + cw])
                    if dma_only:
                        return
                    # two slabs share one full-bank psA tile -> 4-slab pipeline
                    if sl % 2 == 0 or 0 not in ps_pair:
                        ps_pair[0] = psA.tile([128, 2, SLAB_C // 128, NQ],
                                              F32, name="ps")
                    ps = ps_pair[0][:, sl % 2]
                    for u in range(ntile_sl):
                        for k in range(2):
                            nc.tensor.matmul(
                                ps[:, u, :],
                                wbuf[:, k, u * 128:(u + 1) * 128],
                                mlpfT[:, k, :],
                                start=(k == 0), stop=(k == 1))
                    # drains: bias tiles -> bias_sb only; W regions -> pW*
                    u = 0
                    while u < ntile_sl:
                        t = t0 + u
                        if t < T_W1:
                            nc.vector.tensor_scalar_add(
                                bias_sb[:, t, :, :].opt(),
                                ps[:, u, :],
                                bfeat[:, t:t + 1])
                            u += 1
                            continue
                        if t < T_W2:
                            seg = min(ntile_sl - u, T_W2 - t)
                            nc.vector.tensor_tensor(
                                out=pW1[:, t - T_W1:t - T_W1 + seg, :],
                                in0=ps[:, u:u + seg, :],
                                in1=bfeat[:, t:t + seg].unsqueeze(2)
                                .broadcast_to((128, seg, NQ)),
                                op=mybir.AluOpType.add)
                            u += seg
                            continue
                        pt, lo = (pW2, T_W2) if t < T_W3 else (pW3, T_W3)
                        hi = T_W3 if t < T_W3 else N_TILES
                        seg = min(ntile_sl - u, hi - t)
                        nc.vector.tensor_tensor(
                            out=pt[:, :, t - lo:t - lo + seg],
                            in0=ps[:, u:u + seg, :].transpose([0, 2, 1]),
                            in1=bfeat[:, t:t + seg].unsqueeze(1)
                            .broadcast_to((128, NQ, seg)),
                            op=mybir.AluOpType.add)
                        u += seg

                if variant == "dmaonly":
                    for sl in range(N_SLABS):
                        emit_slab(sl, dma_only=True)
                    for g in range(8):
                        xcd = xcpool.tile([128, S, 8, 32], BF, name="xc")
                        nc.scalar.dma_start(xcd[:], xcoord_d[g])
                    for r in range(4):
                        out_row = lpool.tile([128, S, 8, 32], BF,
                                             name="out_rowD", bufs=2)
                        nc.vector.memset(out_row[:], 0.0)
                        for half in range(2):
                            nc.sync.dma_start(out_d[r, half], out_row[64 * half:64 * half + 64, :, :, 0:32])
                    continue

                mlp_on = variant != "nomlp"
                x1s, x2h0s, x2h1s = {}, {}, {}

                # ---- stream head: biases + W1' ----
                for sl in range(0, 1):
                    emit_slab(sl)

                if mlp_on:
                    # b1 rows for all patches: [128(h-part), 4, 8] -> [32, 128]
                    b12 = ppool.tile([128, 2, 4, 8], BF, name="b12")
                    nc.vector.tensor_copy(b12[:], bias_sb[:, 0:2, :, :])
                    psB = psY.tile([NQ, 2, 128], BF, name="psB", tag="y")
                    for h in range(2):
                        nc.tensor.transpose(psB[:, h, :],
                                            b12[:, h, :, :], ident[:])
                    nc.vector.tensor_copy(b1T[:], psB[:])

                for sl in range(1, SL_FIX):
                    emit_slab(sl)

                xcs = {}

                def load_xc(g):
                    xc = xcpool.tile([128, S, 8, 32], BF, name="xc")
                    nc.scalar.dma_start(xc[:], xcoord_d[g])
                    xcs[g] = xc

                def emit_fixup_l1(q):
                    wp = q % 8
                    # statq rows: [0:32 W1^T][32 b1][33:64 garbage][64:128
                    # (W1@w_cd)^T]; moving rows: [em+b_cd][ones][zeros][cd]
                    statq = spool.tile([128, 256], BF, name="statq")
                    if q < 6:  # zero garbage rows once per pool buffer
                        nc.vector.memset(statq[32:64, :], 0.0)
                    nc.scalar.dma_start(statq[32:33, :], b1T[q:q + 1, :, :])
                    psT = psY.tile([32, 256], BF, name="psT", tag="y")
                    for h in range(2):
                        nc.tensor.transpose(
                            psT[:, 128 * h:128 * h + 128],
                            pW1[:, h:h + 63:2, q], ident[:])
                    nc.vector.tensor_copy(statq[0:32, :], psT[:])
                    psP = psY.tile([128, 256], F32, name="psP", tag="y")
                    nc.tensor.matmul(psP[64:128, :], wcde[:], statq[0:32, :],
                                     start=True, stop=True,
                                     tile_position=(0, 64))
                    nc.vector.tensor_copy(statq[64:128, :], psP[64:128, :])
                    y1 = psL1.tile([128, 2, 512], F32, name="y1")
                    xq = xcs[q // 4][:, :, :, 8 * (wp % 4):8 * (wp % 4) + 8]
                    for h in range(2):
                        nc.tensor.matmul(y1[:, h, :],
                                         statq[:, 128 * h:128 * h + 128],
                                         xq, start=True, stop=True)
                    x1 = apool.tile([128, 2, 512], BF, name="x1", bufs=NQ)
                    nc.scalar.activation(
                        x1[:], y1[:], mybir.ActivationFunctionType.Lrelu,
                        scale=1.0, alpha=NEG)
                    x1s[q] = x1

                def emit_l2(q, h, store):
                    x2 = lpool.tile([128, 512], BF, name=f"x2h{h}",
                                    bufs=NQ if h == 0 else H1E)
                    y2 = psY.tile([128, 512], F32, name="y2", tag="y")
                    for k in range(2):
                        nc.tensor.matmul(
                            y2[:], pW2[:, q, 256 * h + 128 * k:
                                        256 * h + 128 * k + 128],
                            x1s[q][:, k, :], start=(k == 0), stop=(k == 1))
                    nc.scalar.activation(
                        x2[:], y2[:], mybir.ActivationFunctionType.Lrelu,
                        bias=bias_sb[:, 2 + h, q // 8, q % 8:q % 8 + 1],
                        scale=1.0, alpha=NEG)
                    store[q] = x2

                def spread(n_items, lo, hi):
                    nsl = hi - lo
                    return {lo + i: list(range(n_items * i // nsl,
                                               n_items * (i + 1) // nsl))
                            for i in range(nsl)}

                fix_sched = spread(NQ, SL_FIX, SL_L2)
                l20_sched = spread(NQ, SL_L2, SL_H1)
                l21_sched = spread(H1E, SL_H1, N_SLABS)

                if mlp_on:
                    load_xc(0)
                    load_xc(1)
                for sl in range(SL_FIX, N_SLABS):
                    emit_slab(sl)
                    if not mlp_on:
                        continue
                    for q in fix_sched.get(sl, ()):
                        if q % 4 == 0 and q // 4 + 2 <= 7:
                            load_xc(q // 4 + 2)
                        emit_fixup_l1(q)
                    for q in l20_sched.get(sl, ()):
                        emit_l2(q, 0, x2h0s)
                    for q in l21_sched.get(sl, ()):
                        emit_l2(q, 1, x2h1s)

                out_state = {}
                bias3all = lpool.tile([128, 4, 4], F32, name="bias3all")
                nc.sync.dma_start(bias3all[0:64, :, :],
                                  bias_sb[0:64, 4, :, 0:4])
                nc.sync.dma_start(bias3all[64:128, :, :],
                                  bias_sb[0:64, 4, :, 4:8])

                def emit_l3(pr):
                    r, po = pr // 4, pr % 4
                    qa, qb = 8 * r + po, 8 * r + po + 4
                    if po == 0:
                        out_state[r] = lpool.tile([128, S, 8, 32], BF,
                                                  name="out_row", bufs=2)
                    y3 = psY.tile([128, S, 8, 8], F32, name="y3", tag="y")
                    for k in range(2):
                        nc.tensor.matmul(
                            y3[0:64, :, :, :],
                            pW3[:, qa, 64 * k:64 * k + 64],
                            x2h0s[qa][:] if k == 0 else x2h1s[qa][:],
                            start=(k == 0), stop=(k == 1))
                    for k in range(2):
                        nc.tensor.matmul(
                            y3[64:128, :, :, :],
                            pW3[:, qb, 64 * k:64 * k + 64],
                            x2h0s[qb][:] if k == 0 else x2h1s[qb][:],
                            start=(k == 0), stop=(k == 1),
                            tile_position=(0, 64))
                    nc.vector.tensor_scalar_add(
                        out_state[r][:, :, :, 8 * po:8 * po + 8], y3[:],
                        bias3all[:, r, po:po + 1])
                    if po == 3:
                        for half in range(2):
                            nc.scalar.dma_start(
                                out_d[r, half],
                                out_state[r][64 * half:64 * half + 64,
                                             :, :, :])

                if variant == "nomlp":
                    for r in range(4):
                        out_row = lpool.tile([128, S, 8, 32], BF,
                                             name="out_rowM", bufs=2)
                        nc.vector.memset(out_row[:], 0.0)
                        for half in range(2):
                            nc.sync.dma_start(out_d[r, half], out_row[64 * half:64 * half + 64])
                if mlp_on:
                    # before l2h1(H1E+i) (slot i), the L3 pair consuming
                    # x2h1[i] must already be emitted (FIFO tag queues)
                    next_pr = [0]

                    def l3_upto(need):
                        while next_pr[0] <= need:
                            emit_l3(next_pr[0])
                            next_pr[0] += 1

                    for i in range(NQ - H1E):
                        l3_upto(4 * (i // 8) + (i % 8) % 4)
                        emit_l2(H1E + i, 1, x2h1s)
                    for pr in range(next_pr[0], NQ // 2):
                        emit_l3(pr)

    nc.compile()
    return nc


def _host_prep(MLP_feature, coord_em, coord_data, w_cd, b_cd, w_feat, b_feat):
    # build the reordered + padded stream:
    # [b1 256][b2 256][b3 64][pad 64][W1' 8192 (c'=i*256+o)][W2][W3]
    j = np.arange(8192)
    w1_perm = (j % 256) * 32 + (j // 256)           # orig c of W1' position j
    w_feat_r = np.zeros((C_PAD, FEAT_CH), np.float32)
    b_feat_r = np.zeros(C_PAD, np.float32)

    def put(dst0, src_idx):
        w_feat_r[dst0:dst0 + len(src_idx)] = w_feat[src_idx]
        b_feat_r[dst0:dst0 + len(src_idx)] = b_feat[src_idx]

    put(0, np.arange(8192, 8448))          # b1
    put(256, np.arange(73984, 74240))      # b2
    put(512, np.arange(90624, 90688))      # b3
    put(640, w1_perm)                      # W1'
    jw2 = np.arange(65536)
    t2, p2 = jw2 // 128, jw2 % 128
    h2, k2, o2 = t2 // 256, (t2 // 128) % 2, t2 % 128
    put(8832, (128 * h2 + o2) * 256 + 128 * k2 + p2 + 8448)      # W2 (h,k,o)
    jw3 = np.arange(16384)
    t3, p3 = jw3 // 128, jw3 % 128
    k3, o3 = t3 // 64, t3 % 64
    put(74368, o3 * 256 + 128 * k3 + p3 + 74240)                 # W3 (k,o)

    wfeatT = np.ascontiguousarray(
        w_feat_r.T.astype(BF16).reshape(2, 128, C_PAD))
    bfeat_t = np.ascontiguousarray(b_feat_r.reshape(N_TILES, 128).T)

    wcde = w_cd.astype(BF16)

    in_maps = []
    for core in range(N_CORES):
        b, hh = core // 2, core % 2
        mlpfT = np.ascontiguousarray(
            MLP_feature[b, :, 4 * hh:4 * hh + 4, :].reshape(2, 128, NQ)
            .transpose(1, 0, 2)).astype(BF16)
        em = coord_em[b].reshape(IN_CH, S, OUT_H, OUT_W)[
            :, :, 32 * hh:32 * hh + 32, :]
        cd = coord_data[b].reshape(S, OUT_H, OUT_W, OUT_CH)[
            :, 32 * hh:32 * hh + 32]
        xcoord = np.zeros((128, S, 32, OUT_W), BF16)
        xcoord[:32] = (em + b_cd[:, None, None, None]).astype(BF16)
        xcoord[32] = 1.0
        xcoord[64:] = cd.transpose(3, 0, 1, 2).astype(BF16)
        xcg = np.empty((8, 128, S, 8, 32), BF16)
        for g in range(8):
            r, wh = g // 2, g % 2
            xcg[g] = xcoord[:, :, 8 * r:8 * r + 8, 32 * wh:32 * wh + 32]
        in_maps.append({
            "wfeatT": wfeatT, "bfeat": bfeat_t, "mlpfT": mlpfT,
            "xcoord": xcg, "wcde": wcde,
        })
    return in_maps


def kernel(**inputs):
    inputs = {k: np.asarray(v) for k, v in inputs.items()}
    if "nc" not in _CACHE:
        _CACHE["nc"] = _build()
    nc = _CACHE["nc"]
    in_maps = _host_prep(**inputs)
    res = run_bass_kernel_spmd(nc, in_maps, core_ids=list(range(N_CORES)))
    out = np.empty((B, OUT_CH, S, OUT_H, OUT_W), np.float32)
    for core in range(N_CORES):
        b, hh = core // 2, core % 2
        o = res.results[core]["out"].astype(np.float32)  # [4, 2, 64, S, 8, 32]
        for r in range(4):
            for half in range(2):
                out[b, :, :, 32 * hh + 8 * r:32 * hh + 8 * r + 8,
                    32 * half:32 * half + 32] = o[r, half]
    return out
